# revision 1
# baseline (speedup 1.0000x reference)
"""Trainium2 Bass kernel for nn_BasePBC (PBC tap products).

Math:
  Reference computes, for each tap s=(m,n) with |m*n|<=25, |m|,|n|<=25:
      En  = roll(E, n); Emn = roll(E, m+n); Em = roll(E, m)   (roll along W)
      A   = En * conj(Emn);  Asum = A + flip_modes(A);  F = Asum * Em
  Key identities used here:
      roll(E,n)*conj(roll(E,m+n)) = roll(C_m, n) with C_m = E*conj(roll(E,m))
      Asum(mode0) == Asum(mode1) == roll(B_m, n),  B_m = sum_mu C_m[mu]
  So per tap:  F_mu[w] = B_m[w-n] * E_mu[w-m]   -- only 51 distinct B_m.

Distribution (SPMD, 8 cores, identical program):
  Shard W into 8 slices of 2048. Each core computes ALL 449 taps on its
  slice. Per-core differences live purely in the input data (a haloed
  window of E). On-chip layout puts (tap,b) rows on the 128 partitions;
  circular shifts become per-row flat-element offsets realized with
  indirect (gather) DMAs from DRAM using constant offset tables.
"""

import numpy as np

import concourse.bass as bass
import concourse.bacc as bacc
import concourse.mybir as mybir
from concourse.tile import TileContext

# ---------------- problem constants (must match reference.py) --------------
RHO, L = 1.0, 50
TAPS = [
    (m, n)
    for m in range(-L // 2, L // 2 + 1)
    for n in range(-L // 2, L // 2 + 1)
    if abs(m * n) <= RHO * L // 2
]
S = len(TAPS)  # 449
B, W, NMODES = 2, 16384, 2
NCORES = 8
WLOC = W // NCORES  # 2048
EHALO = 64  # halo on each side of the local E window
EW = WLOC + 2 * EHALO  # 2176: e-plane row width
MS = sorted({m for m, _ in TAPS})  # -25..25
NM = len(MS)  # 51
M_IDX = {m: i for i, m in enumerate(MS)}
BMH = 32  # B_m halo (covers |n| <= 25)
BMW = WLOC + 2 * BMH  # 2112
NROWS = S * B  # 898   (row r = t*2 + b)
NB = 8
BR = 128  # rows per block; large DMAs must span all 128 partitions
#           (partial-partition stores run ~12x slower). 7 full blocks + one
#           2-row tail whose tiny store can afford the slow path.
NMB_PAD = 128  # B_m rows padded from 102 to 128 for the same reason
NCOLS = 2 + 2 * NB  # offset-table columns (merged gathers)
# Merged-gather column layouts (fp16 elements):
#   bm merged row:  Ar @ [0:2048],   Ai @ [2112:4160]           (run 4224)
#   e  merged row:  er0 @ [0:2048],  ei0 @ [2176:4224],
#                   er1 @ [4352:6400], ei1 @ [6528:8576]        (run 8576)
#   bm-phase e row: 4 planes, 2112 wide each, starts 0/2176/4352/6528 (run 8640)
ERUN = 3 * EW + WLOC  # 8576
BRUN = 2 * BMW  # 4224
URUN = 3 * EW + BMW  # 8640

FP = mybir.dt.float16
NPFP = np.float16


def _pidx(b, mu, ri):
    return (b * 2 + mu) * 2 + ri


def _build_offsets() -> np.ndarray:
    offs = np.zeros((128, NCOLS), dtype=np.int32)
    # --- B_m phase (col 0: unshifted 4-plane run; col 1: shifted by m) ---
    for mi, m in enumerate(MS):
        for b in range(B):
            r = mi * 2 + b
            base = _pidx(b, 0, 0) * EW
            offs[r, 0] = base + (EHALO - BMH)
            offs[r, 1] = base + (EHALO - BMH) - m
    # --- F phase (cols 2 + k*2: merged bm run; +1: merged e run) ---
    for k in range(NB):
        r0 = k * BR
        for p in range(BR):
            r = r0 + p
            if r >= NROWS:
                break
            t, b = r // 2, r % 2
            m, n = TAPS[t]
            c0 = 2 + k * 2
            bmrow = M_IDX[m] * 2 + b
            offs[p, c0 + 0] = (bmrow * 2) * BMW + BMH - n  # Ar..Ai run
            offs[p, c0 + 1] = _pidx(b, 0, 0) * EW + EHALO - m  # er0..ei1 run
    return offs


def _build_nc(reps: int = 1):
    nc = bacc.Bacc("TRN2", debug=False, target_bir_lowering=False)
    e_dram = nc.dram_tensor("e_planes", [8, EW], FP, kind="ExternalInput")
    offs_dram = nc.dram_tensor("offs", [128, NCOLS], mybir.dt.int32, kind="ExternalInput")
    out_dram = nc.dram_tensor("out", [NROWS, 2, 2, WLOC], FP, kind="ExternalOutput")
    bm_dram = nc.dram_tensor("bm_scratch", [NMB_PAD, 2, BMW], FP)  # Internal scratch

    NMB = NMB_PAD  # padded to 128 partitions
    with TileContext(nc) as tc:
        with tc.tile_pool(name="const", bufs=1) as cpool:
            offs = cpool.tile([128, NCOLS], mybir.dt.int32)
            nc.sync.dma_start(out=offs[:], in_=offs_dram[:])
            for _rep in range(reps):
                _emit_body(nc, tc, offs, e_dram, bm_dram, out_dram, NMB)
    nc.compile()
    return nc


def _emit_body(nc, tc, offs, e_dram, bm_dram, out_dram, NMB):
    if True:
        if True:
            # ---------------- B_m phase ----------------
            with tc.tile_pool(name="bmph", bufs=1) as bpool:
                gu = bpool.tile([NMB, URUN], FP, tag="bmgu", name="bmgu")
                gs = bpool.tile([NMB, URUN], FP, tag="bmgs", name="bmgs")
                for t_, j in ((gu, 0), (gs, 1)):
                    nc.gpsimd.indirect_dma_start(
                        out=t_[:],
                        out_offset=None,
                        in_=e_dram[:],
                        in_offset=bass.IndirectOffsetOnAxis(
                            ap=offs[:NMB, j : j + 1], axis=1
                        ),
                    )
                ur0, ui0, ur1, ui1 = (
                    gu[:, 0:BMW],
                    gu[:, EW : EW + BMW],
                    gu[:, 2 * EW : 2 * EW + BMW],
                    gu[:, 3 * EW : 3 * EW + BMW],
                )
                sr0, si0, sr1, si1 = (
                    gs[:, 0:BMW],
                    gs[:, EW : EW + BMW],
                    gs[:, 2 * EW : 2 * EW + BMW],
                    gs[:, 3 * EW : 3 * EW + BMW],
                )
                bm = bpool.tile([NMB, 2, BMW], FP, tag="bm")
                tp = [bpool.tile([NMB, BMW], FP, tag=f"bmt{i}", name=f"bmt{i}") for i in range(4)]
                V = nc.vector
                # real part: sum_mu (ur*sr + ui*si)
                V.tensor_mul(out=tp[0][:], in0=ur0, in1=sr0)
                V.tensor_mul(out=tp[1][:], in0=ui0, in1=si0)
                V.tensor_mul(out=tp[2][:], in0=ur1, in1=sr1)
                V.tensor_mul(out=tp[3][:], in0=ui1, in1=si1)
                V.tensor_add(out=tp[0][:], in0=tp[0][:], in1=tp[1][:])
                V.tensor_add(out=tp[2][:], in0=tp[2][:], in1=tp[3][:])
                V.tensor_add(out=bm[:, 0, :], in0=tp[0][:], in1=tp[2][:])
                # imag part: sum_mu (ui*sr - ur*si)
                V.tensor_mul(out=tp[0][:], in0=ui0, in1=sr0)
                V.tensor_mul(out=tp[1][:], in0=ur0, in1=si0)
                V.tensor_mul(out=tp[2][:], in0=ui1, in1=sr1)
                V.tensor_mul(out=tp[3][:], in0=ur1, in1=si1)
                V.tensor_sub(out=tp[0][:], in0=tp[0][:], in1=tp[1][:])
                V.tensor_sub(out=tp[2][:], in0=tp[2][:], in1=tp[3][:])
                V.tensor_add(out=bm[:, 1, :], in0=tp[0][:], in1=tp[2][:])
                nc.sync.dma_start(out=bm_dram[:], in_=bm[:])

            # ---------------- F phase ----------------
            with (
                tc.tile_pool(name="fop", bufs=3) as fpool,
                tc.tile_pool(name="ftmp", bufs=4) as tpool,
                tc.tile_pool(name="fout", bufs=3) as opool,
            ):
                for k in range(NB):
                    r0 = k * BR
                    br = min(BR, NROWS - r0)  # last block: 2 rows (tiny
                    # partial-partition store, ~32KB — negligible)
                    c0 = 2 + k * 2
                    gbm = fpool.tile([128, BRUN], FP, tag="gbm", name="gbm")
                    ge = fpool.tile([128, ERUN], FP, tag="ge", name="ge")
                    for t_, jj, srct in ((gbm, 0, bm_dram), (ge, 1, e_dram)):
                        nc.gpsimd.indirect_dma_start(
                            out=t_[:br],
                            out_offset=None,
                            in_=srct[:],
                            in_offset=bass.IndirectOffsetOnAxis(
                                ap=offs[:br, c0 + jj : c0 + jj + 1],
                                axis=len(srct.shape) - 1,
                            ),
                        )
                    ar, ai = gbm[:br, 0:WLOC], gbm[:br, BMW : BMW + WLOC]
                    er0, ei0 = ge[:br, 0:WLOC], ge[:br, EW : EW + WLOC]
                    er1, ei1 = (
                        ge[:br, 2 * EW : 2 * EW + WLOC],
                        ge[:br, 3 * EW : 3 * EW + WLOC],
                    )
                    f = opool.tile([128, 2, 2, WLOC], FP, tag="f")
                    V = nc.vector
                    for mu, (er, ei) in enumerate([(er0, ei0), (er1, ei1)]):
                        p = tpool.tile([128, WLOC], FP, tag="p", name="p")
                        q = tpool.tile([128, WLOC], FP, tag="q", name="q")
                        V.tensor_mul(out=p[:br], in0=ar, in1=er)
                        V.tensor_mul(out=q[:br], in0=ai, in1=ei)
                        V.tensor_sub(out=f[:br, mu, 0, :], in0=p[:br], in1=q[:br])
                        p2 = tpool.tile([128, WLOC], FP, tag="p2", name="p2")
                        q2 = tpool.tile([128, WLOC], FP, tag="q2", name="q2")
                        V.tensor_mul(out=p2[:br], in0=ar, in1=ei)
                        V.tensor_mul(out=q2[:br], in0=ai, in1=er)
                        V.tensor_add(out=f[:br, mu, 1, :], in0=p2[:br], in1=q2[:br])
                    nc.sync.dma_start(out=out_dram[r0 : r0 + br], in_=f[:br])


# ---------------- host side: cached compiled executable --------------------
_CACHE: dict = {}


def _get_runner(reps: int = 1):
    """Build nc once per reps and wrap a cached jitted SPMD executor
    (modeled on concourse.bass2jax.run_bass_via_pjrt, reusable across
    calls). reps>1 repeats the kernel body inside the NEFF (for timing)."""
    key = ("runner", reps)
    if key in _CACHE:
        return _CACHE[key]

    import jax
    from jax.sharding import Mesh, PartitionSpec
    from jax.experimental.shard_map import shard_map
    from concourse import bass2jax

    nc = _build_nc(reps)
    bass2jax.install_neuronx_cc_hook()

    partition_name = nc.partition_id_tensor.name if nc.partition_id_tensor else None
    in_names, out_names, out_avals = [], [], []
    for alloc in nc.m.functions[0].allocations:
        if not isinstance(alloc, mybir.MemoryLocationSet):
            continue
        name = alloc.memorylocations[0].name
        if alloc.kind == "ExternalInput":
            if name != partition_name:
                in_names.append(name)
        elif alloc.kind == "ExternalOutput":
            out_names.append(name)
            out_avals.append(
                jax.core.ShapedArray(tuple(alloc.tensor_shape), mybir.dt.np(alloc.dtype))
            )
    n_params = len(in_names)
    n_outs = len(out_avals)
    all_in_names = list(in_names) + list(out_names)
    if partition_name is not None:
        all_in_names.append(partition_name)
    donate = tuple(range(n_params, n_params + n_outs))

    def _body(*args):
        operands = list(args)
        if partition_name is not None:
            operands.append(bass2jax.partition_id_tensor())
        outs = bass2jax._bass_exec_p.bind(
            *operands,
            out_avals=tuple(out_avals),
            in_names=tuple(all_in_names),
            out_names=tuple(out_names),
            lowering_input_output_aliases=(),
            sim_require_finite=True,
            sim_require_nnan=True,
            nc=nc,
        )
        return tuple(outs)

    devices = jax.devices()[:NCORES]
    assert len(devices) == NCORES
    mesh = Mesh(np.asarray(devices), ("core",))
    in_specs = (PartitionSpec("core"),) * (n_params + n_outs)
    out_specs = (PartitionSpec("core"),) * n_outs
    smapped = shard_map(
        _body, mesh=mesh, in_specs=in_specs, out_specs=out_specs, check_rep=False
    )
    sharded = jax.jit(smapped, donate_argnums=donate, keep_unused=True)

    class Runner:
        pass

    R = Runner()
    R.sharded_nodonate = jax.jit(smapped, keep_unused=True)
    R.in_names, R.out_names, R.out_avals, R.mesh = in_names, out_names, out_avals, mesh

    def run(in_maps, device_only=False):
        concat_in = [
            np.concatenate([np.asarray(in_maps[c][nm]) for c in range(NCORES)], axis=0)
            for nm in in_names
        ]
        concat_zeros = [
            np.zeros((NCORES * av.shape[0], *av.shape[1:]), av.dtype) for av in out_avals
        ]
        out_arrs = sharded(*concat_in, *concat_zeros)
        if device_only:
            for o in out_arrs:
                o.block_until_ready()
            return None
        return [
            {
                nm: np.asarray(out_arrs[i]).reshape(NCORES, *out_avals[i].shape)[c]
                for i, nm in enumerate(out_names)
            }
            for c in range(NCORES)
        ]

    R.run = run
    _CACHE[key] = R
    return R


def _make_in_maps(E_real: np.ndarray, E_imag: np.ndarray):
    offs = _CACHE.get("offs")
    if offs is None:
        offs = _CACHE["offs"] = _build_offsets()
    E_real = np.asarray(E_real, dtype=np.float32)
    E_imag = np.asarray(E_imag, dtype=np.float32)
    in_maps = []
    for c in range(NCORES):
        idx = np.arange(c * WLOC - EHALO, (c + 1) * WLOC + EHALO) % W
        planes = np.empty((8, EW), dtype=NPFP)
        for b in range(B):
            for mu in range(NMODES):
                planes[_pidx(b, mu, 0)] = E_real[b, idx, mu].astype(NPFP)
                planes[_pidx(b, mu, 1)] = E_imag[b, idx, mu].astype(NPFP)
        in_maps.append({"e_planes": planes, "offs": offs})
    return in_maps


def _assemble(results) -> np.ndarray:
    out = np.empty((B, W, NMODES, S), dtype=np.complex64)
    for c in range(NCORES):
        o = results[c]["out"][:NROWS].astype(np.float32).reshape(S, B, 2, 2, WLOC)
        cx = o[:, :, :, 0, :] + 1j * o[:, :, :, 1, :]  # [S, B, mu, WLOC]
        out[:, c * WLOC : (c + 1) * WLOC, :, :] = cx.transpose(1, 3, 2, 0)
    return out


def kernel(E_real: np.ndarray, E_imag: np.ndarray) -> np.ndarray:
    R = _get_runner()
    in_maps = _make_in_maps(E_real, E_imag)
    return _assemble(R.run(in_maps))


def _timed_loop(fn, args, n):
    import time
    import jax

    t0 = time.perf_counter()
    outs = [fn(*args) for _ in range(n)]
    jax.block_until_ready(outs)
    return time.perf_counter() - t0


def _device_args(R, E_real, E_imag):
    import jax
    from jax.sharding import NamedSharding, PartitionSpec

    in_maps = _make_in_maps(E_real, E_imag)
    concat_in = [
        np.concatenate([np.asarray(in_maps[c][nm]) for c in range(NCORES)], axis=0)
        for nm in R.in_names
    ]
    concat_zeros = [
        np.zeros((NCORES * av.shape[0], *av.shape[1:]), av.dtype) for av in R.out_avals
    ]
    shard = NamedSharding(R.mesh, PartitionSpec("core"))
    return [jax.device_put(a, shard) for a in (*concat_in, *concat_zeros)]


def bench(E_real: np.ndarray, E_imag: np.ndarray, iters: int = 40, hi_reps: int = 9):
    """Estimate on-device kernel time by differencing NEFFs with the body
    repeated 1x vs hi_reps inside a single execution (cancels per-call
    dispatch overhead through the tunnel). Returns (sec_per_kernel, None)."""
    import jax

    times = {}
    for reps in (1, hi_reps):
        R = _get_runner(reps)
        args = _device_args(R, E_real, E_imag)
        fn = R.sharded_nodonate
        jax.block_until_ready(fn(*args))  # compile+warm
        _timed_loop(fn, args, 3)
        best = min(_timed_loop(fn, args, iters) / iters for _ in range(3))
        times[reps] = best
        print(f"  reps={reps}: per-exec {best * 1e6:.0f} us")
    per_kernel = (times[hi_reps] - times[1]) / (hi_reps - 1)
    return per_kernel, None



# revision 2
# speedup vs baseline: 5.1304x; 5.1304x over previous
"""Trainium2 Bass kernel for nn_BasePBC (PBC tap products), v3.

Math: F_mu[w] = B_m[w-n] * E_mu[w-m], B_m = sum_mu E_mu*conj(roll(E_mu,m)),
449 taps (m,n), 51 distinct m, B=2 batches, 2 modes, W=16384.

Key techniques:
  * Interleaved-complex fp16; hand-built DVE uop programs CMUL/CMULC do a
    full complex multiply per cycle/lane in 2x_1P packed mode (fp32
    internal, single rounding).
  * "Chunked rows": an SBUF row holds ONE gathered run of B_m; the K
    consecutive-n windows of that run are read as overlapping views via a
    [+2, K] access-pattern dim, and the E factor is broadcast across the
    K windows with a stride-0 dim. This cuts the replicated E/B traffic
    by ~4x vs one-row-per-tap.
  * Two F blocks (chunk sizes K_A/K_B) packing all (b, m, chunk) rows
    into <=128 partitions each; full-width indirect gathers spread across
    all 16 SDMA engines.
  * W sharded over 8 cores (2048 complex each + halos), SPMD.
"""

import numpy as np

import concourse.bass as bass
import concourse.bacc as bacc
import concourse.mybir as mybir
from concourse.ap import AP
from concourse.tile import TileContext

# ---------------- problem constants (must match reference.py) --------------
RHO, L = 1.0, 50
TAPS = [
    (m, n)
    for m in range(-L // 2, L // 2 + 1)
    for n in range(-L // 2, L // 2 + 1)
    if abs(m * n) <= RHO * L // 2
]
S = len(TAPS)  # 449
B, W, NMODES = 2, 16384, 2
NCORES = 8
WLOC = W // NCORES  # 2048 complex
EHALO = 64
EW = WLOC + 2 * EHALO  # 2176 complex
PW = 2 * EW  # 4352 fp16 per plane row
BMH = 32
BMW = WLOC + 2 * BMH  # 2112 complex
BMF = 2 * BMW  # 4224 fp16
WF = 2 * WLOC  # 4096 fp16
MS = sorted({m for m, _ in TAPS})
NM = len(MS)  # 51
NBM = B * NM  # 102

FP = mybir.dt.float16
NPFP = np.float16
NREP = 4  # one DRAM replica of e_planes PER GATHER (defeats HBM hotspotting)
ESTRIDE = 1 << 21  # elements between replicas (4 MB): spreads HBM banks
BMSTRIDE = 1 << 19  # elements between the two bm copies (1 MB)

_CNT = {m: len([1 for mm, _ in TAPS if mm == m]) for m in MS}
_NMAX = {m: max(n for mm, n in TAPS if mm == m) for m in MS}
_NMIN = {m: min(n for mm, n in TAPS if mm == m) for m in MS}


def _rbm(b, m):
    return b * NM + (25 - m)


# ---- chunking search: assign each m to block A or B, chunk sizes K_A/K_B ---
def _block_rows(K, ms):
    """rows (b,m,chunk) for chunk size K over the given m set."""
    rows = []
    for b in range(B):
        for m in ms:
            cnt, nmax, nmin = _CNT[m], _NMAX[m], _NMIN[m]
            nch = -(-cnt // K)
            for c in range(nch):
                n_hi = nmax - c * K
                if n_hi - K + 1 < nmin:  # clamp last chunk into range
                    n_hi = max(nmin + K - 1, n_hi) if cnt >= K else nmax
                    n_hi = min(n_hi, nmax)
                rows.append((b, m, n_hi))
    return rows


def _search_blocks():
    """Pick chunk sizes (K_A, K_B) and an m->block assignment minimizing
    estimated max(DMA time, vector time). Vector time ~ (K_A+K_B); DMA
    time ~ bytes moved. Both blocks must fit in 128 partitions (x2 for b).
    """
    fixed_bytes = (102 * (PW + BMF) + 102 * 2 * BMF + 102 * BMF) * 2 + 4096

    best = None
    for KA in range(3, 10):
        for KB in range(2, KA + 1):
            # bytes per m if assigned to block X (out windows + esel + bmn)
            def _bm(m, K):
                rows = -(-_CNT[m] // K)
                return rows * (K * 2 * WF * 2 + (PW + WF) * 2 + (WF + 2 * (K - 1)) * 2)

            def _rows(m, K):
                return 2 * -(-_CNT[m] // K)

            # DP over m: state = rows used in A (0..128)
            INF = float("inf")
            dp = {0: 0.0}
            choice: list[dict[int, int]] = []
            rows_tot = 0
            for m in MS:
                nd = {}
                ch = {}
                ra, rb = _rows(m, KA), _rows(m, KB)
                ba, bb = _bm(m, KA), _bm(m, KB)
                for ua, cost in dp.items():
                    if ua + ra <= 128 and (cost + ba) < nd.get(ua + ra, INF):
                        nd[ua + ra] = cost + ba
                        ch[ua + ra] = 1
                    if (cost + bb) < nd.get(ua, INF):
                        nd[ua] = cost + bb
                        ch[ua] = 0
                dp = nd
                choice.append(ch)
            # recover best end state with rows_B <= 128
            bestu = None
            for ua, cost in dp.items():
                # compute rows_B by replay later; first need feasibility:
                pass
            # replay all end states
            for ua, cost in sorted(dp.items(), key=lambda x: x[1]):
                # reconstruct assignment
                A, Bm = [], []
                u = ua
                ok = True
                for i in range(len(MS) - 1, -1, -1):
                    m = MS[i]
                    c = choice[i].get(u)
                    if c is None:
                        ok = False
                        break
                    if c == 1:
                        A.append(m)
                        u -= _rows(m, KA)
                    else:
                        Bm.append(m)
                if not ok or u != 0:
                    continue
                rb_used = sum(_rows(m, KB) for m in Bm)
                if rb_used > 128 or not A or not Bm:
                    continue
                dma_ns = (cost + fixed_bytes) / 345.0  # GB/s -> ns/B
                vec_ns = (2 * (KA + KB) * 2048 + 4 * 58 + 6600) / 0.96
                est = max(dma_ns, vec_ns) + 0.2 * min(dma_ns, vec_ns)
                if best is None or est < best[0]:
                    ra_list = _block_rows(KA, sorted(A))
                    rb_list = _block_rows(KB, sorted(Bm))
                    if len(ra_list) <= 128 and len(rb_list) <= 128:
                        best = (est, KA, KB, ra_list, rb_list)
                break  # only cheapest feasible end state per (KA,KB)
    assert best is not None
    return best[1], best[2], best[3], best[4]


K_A, K_B, ROWS_A, ROWS_B = _search_blocks()
R_A, R_B = len(ROWS_A), len(ROWS_B)
RL_A = WF + 2 * (K_A - 1)  # gathered bm run per row
RL_B = WF + 2 * (K_B - 1)
ERUN = PW + WF  # gathered e run per row (covers both planes)
for b, m, n_hi in ROWS_A + ROWS_B:
    K = K_A if (b, m, n_hi) in set(ROWS_A) else K_B
for rows, K in ((ROWS_A, K_A), (ROWS_B, K_B)):
    for b, m, n_hi in rows:
        assert 2 * (BMH - n_hi + K - 1) + WF <= BMF, (m, n_hi, K)
        assert BMH - n_hi >= 0

# ======================= custom packed-complex DVE ops =====================
from concourse.dve_uop import (
    DveOpSpec,
    OpConfig,
    UopConfig,
    UopDpConfig,
    InpSel,
    AluOp,
    AluInp,
    DelayInp,
    OutSel,
    OutPath,
    Trigger,
    ENABLE,
)
from concourse.dve_spec import Spec, Src0, Src1
from concourse import dve_ops as _DOPS


def _pk_view(x):
    x = np.asarray(x)
    x = x.reshape(x.shape[0], -1)
    return x[:, 0::2].astype(np.float32), x[:, 1::2].astype(np.float32)


def _pk_join(fr, fi):
    out = np.empty((fr.shape[0], 2 * fr.shape[1]), np.float32)
    out[:, 0::2] = fr
    out[:, 1::2] = fi
    return out


def _ref_cmul(in0, in1, c0, c1, c2):
    ar, ai = _pk_view(in0)
    br, bi = _pk_view(in1)
    return _pk_join(ar * br - ai * bi, ar * bi + ai * br)


def _ref_cmulc(in0, in1, c0, c1, c2):
    ar, ai = _pk_view(in0)
    br, bi = _pk_view(in1)
    return _pk_join(ar * br + ai * bi, ai * br - ar * bi)


def _cmul_uop(conj: bool) -> UopConfig:
    u = UopConfig()
    u.enable_input(InpSel.SRC_0, 0)
    u.enable_input(InpSel.SRC_0_HI, 1)
    u.enable_input(InpSel.SRC_1, 2)
    u.enable_input(InpSel.SRC_1_HI, 3)
    dp = u.datapath_config
    dp[0] = (
        UopDpConfig()
        .enable_alu(AluOp.MULTIPLY, AluInp.PREV_ALU_OUT, AluInp.PREV_DELAY_1)
        .enable_delay_from_src(DelayInp.PREV_ALU_OUT, 3)
        .pass_through_delay(0, 1, 2)
    )
    dp[1] = (
        UopDpConfig()
        .enable_alu(AluOp.MULTIPLY, AluInp.PREV_DELAY_0, AluInp.PREV_DELAY_2)
        .enable_delay_from_src(DelayInp.PREV_ALU_OUT, 4)
        .pass_through_delay(0, 1, 2, 3)
    )
    dp[2] = (
        UopDpConfig()
        .enable_alu(
            AluOp.ADD if conj else AluOp.SUBTRACT,
            AluInp.PREV_DELAY_4,
            AluInp.PREV_ALU_OUT,
        )
        .pass_through_delay(0, 1, 2, 3)
    )
    dp[3] = (
        UopDpConfig()
        .enable_alu(AluOp.MULTIPLY, AluInp.PREV_DELAY_3, AluInp.PREV_DELAY_2)
        .enable_delay_from_src(DelayInp.PREV_ALU_OUT, 4)
        .pass_through_delay(0, 1)
    )
    dp[4] = (
        UopDpConfig()
        .enable_alu(AluOp.MULTIPLY, AluInp.PREV_DELAY_0, AluInp.PREV_DELAY_1)
        .enable_delay_from_src(DelayInp.PREV_ALU_OUT, 0)
        .pass_through_delay(4)
    )
    dp[5] = (
        UopDpConfig()
        .enable_alu(
            AluOp.SUBTRACT if conj else AluOp.ADD,
            AluInp.PREV_ALU_OUT,
            AluInp.PREV_DELAY_0,
        )
        .pass_through_delay(4)
    )
    dp[6] = UopDpConfig().pass_through_alu().pass_through_delay(4)
    dp[7] = UopDpConfig().pass_through_alu().pass_through_delay(4)
    u.require_inp0 = ENABLE
    u.require_inp1 = ENABLE
    u.trigger = (Trigger.SRC_TENSOR_DONE, Trigger.NONE, Trigger.NONE)
    u.next_uop = (0, 0, 0)
    u.enable_output(OutSel.DELAY_4, OutPath.WR0_LO)
    u.enable_output(OutSel.ALU_OUT, OutPath.WR0_HI)
    return u


class _HandDveOp:
    def __init__(self, name, conj, reference):
        self.name = name
        self.subdim = False
        self.spec = Spec(body=Src0 * Src1, reference=reference)
        self._conj = conj
        self._cache = {}

    def compile(self, ver):
        if ver not in self._cache:
            s = DveOpSpec(
                name=self.name,
                opcode=_DOPS.get_dve_sub_opcode(self.name),
                uops=[_cmul_uop(self._conj)],
                uops_2x=[_cmul_uop(self._conj)],
                op=OpConfig(),
                perf_max=1,
                rd1_en=True,
            )
            s.validate(ver)
            self._cache[ver] = s
        return self._cache[ver]


def _register_ops():
    by_name = {op.name: op for op in _DOPS.OPS}
    out = {}
    for name, conj, ref in (
        ("CMUL_PK_ANT", False, _ref_cmul),
        ("CMULC_PK_ANT", True, _ref_cmulc),
    ):
        if name not in by_name:
            _DOPS._SUB_OPCODE_FOR_NAME[name] = _DOPS._CUSTOM_DVE_ROW_BASE + len(
                _DOPS.OPS
            )
            op = _HandDveOp(name, conj, ref)
            _DOPS.OPS.append(op)
            _DOPS.CUSTOM_DVE_SPECS[name] = op.spec
            out[name] = op
        else:
            out[name] = by_name[name]
    return out


_OPS = _register_ops()
CMUL = _OPS["CMUL_PK_ANT"]
CMULC = _OPS["CMULC_PK_ANT"]


def _emit_pk(nc, op, out_ap, in0_ap, in1_ap):
    bi = nc.vector._custom_dve(op, out=out_ap, in0=in0_ap, in1=in1_ap)
    bi.ins.perf_max = 1
    return bi


# ----------------------------- offsets table -------------------------------
def _build_offsets() -> np.ndarray:
    offs = np.zeros((128, 6), dtype=np.int32)
    EREP = ESTRIDE  # replica stride in elements (bank-spread)
    for col, (rows, K), bmrep, erep in (
        (0, (ROWS_A, K_A), 0, 2),
        (2, (ROWS_B, K_B), 1, 3),
    ):
        for j, (b, m, n_hi) in enumerate(rows):
            offs[j, col] = bmrep * BMSTRIDE + _rbm(b, m) * BMF + 2 * (BMH - n_hi)
            offs[j, col + 1] = erep * EREP + b * 2 * PW + 2 * (EHALO - m)
    for j in range(NBM):  # row j = b*NM + (25 - m)
        b, m = j // NM, 25 - (j % NM)
        offs[j, 4] = 0 * EREP + b * 2 * PW + 2 * (EHALO - BMH)  # gu
        offs[j, 5] = 1 * EREP + b * 2 * PW + 2 * (EHALO - BMH - m)  # gs
    return offs


# ============================ device kernel ================================
def _build_nc(reps: int = 1):
    nc = bacc.Bacc("TRN2", debug=False, target_bir_lowering=False)
    e_dram = nc.dram_tensor("e_planes", [2 * B, PW], FP, kind="ExternalInput")
    e_spread = nc.dram_tensor("e_spread", [NREP, ESTRIDE], FP)
    offs_dram = nc.dram_tensor("offs", [128, 6], mybir.dt.int32, kind="ExternalInput")
    out_a = nc.dram_tensor("out_a", [R_A, 2, K_A, WF], FP, kind="ExternalOutput")
    out_b = nc.dram_tensor("out_b", [R_B, 2, K_B, WF], FP, kind="ExternalOutput")
    bm_dram = nc.dram_tensor("bm_scratch", [2, BMSTRIDE], FP)
    with TileContext(nc) as tc:
        with tc.tile_pool(name="const", bufs=1) as cpool:
            offs = cpool.tile([128, 6], mybir.dt.int32)
            nc.sync.dma_start(out=offs[:], in_=offs_dram[:])
            for _rep in range(reps):
                _emit_body(nc, tc, offs, e_dram, e_spread, bm_dram, out_a, out_b)
    nc.compile()
    return nc


def _emit_body(nc, tc, offs, e_dram, e_spread, bm_dram, out_a, out_b):
    def _gather(out_ap, src, col, R):
        nc.gpsimd.indirect_dma_start(
            out=out_ap,
            out_offset=None,
            in_=src[:],
            in_offset=bass.IndirectOffsetOnAxis(ap=offs[:R, col : col + 1], axis=1),
        )

    def _pk3(t_ap, two_plane_width):
        """[[pitch, R], [PW, 2], [1, w]] view of a [128, >=PW+w] tile."""
        p = t_ap.ap[0][0]
        return lambda R, off=0: AP(
            t_ap.tensor, t_ap.offset + off, [[p, R], [PW, 2], [1, two_plane_width]]
        )

    with (
        tc.tile_pool(name="bmph", bufs=1) as bpool,
        tc.tile_pool(name="fin", bufs=2) as ipool,
        tc.tile_pool(name="fout", bufs=6) as opool,
    ):
        # seed bank-spread DRAM replicas of the e planes (one DMA,
        # stride-0 DRAM source broadcast)
        nc.sync.dma_start(
            out=AP(e_spread[:].tensor, 0, [[ESTRIDE, NREP], [1, 2 * B * PW]]),
            in_=AP(e_dram[:].tensor, 0, [[0, NREP], [1, 2 * B * PW]]),
        )

        # ---------------- B_m phase (gathers first in ring order) --------
        gu = bpool.tile([128, PW + BMF], FP, tag="gu", name="gu")
        gs = bpool.tile([128, PW + BMF], FP, tag="gs", name="gs")
        _gather(gu[:NBM], e_spread, 4, NBM)
        _gather(gs[:NBM], e_spread, 5, NBM)

        # Prefetch block A's E gather (overlaps the B_m compute); block B's
        # is issued later so its packets interleave with block A's stores.
        esel_a = ipool.tile([128, ERUN], FP, tag="esel", name="esel")
        _gather(esel_a[:R_A], e_spread, 1, R_A)

        t = bpool.tile([128, 2, BMF], FP, tag="bmt", name="bmt")
        gu3 = _pk3(gu[:, :], BMF)(NBM)
        gs3 = _pk3(gs[:, :], BMF)(NBM)
        _emit_pk(nc, CMULC, t[:NBM, :, :], gu3, gs3)
        bm = bpool.tile([128, BMF], FP, tag="bm", name="bm")
        nc.vector.tensor_add(out=bm[:NBM], in0=t[:NBM, 0, :], in1=t[:NBM, 1, :])
        for r, eng in ((0, nc.sync), (1, nc.scalar)):
            eng.dma_start(
                out=AP(bm_dram[:].tensor, r * BMSTRIDE, [[BMF, NBM], [1, BMF]]),
                in_=bm[:NBM],
            )

        # ---------------- F phase: two chunked blocks ----------------
        st = 0
        esel_b = None
        for bi, (rows, K, RL, col, out_d, esel) in enumerate(
            (
                (ROWS_A, K_A, RL_A, 0, out_a, esel_a),
                (ROWS_B, K_B, RL_B, 2, out_b, None),
            )
        ):
            R = len(rows)
            if esel is None:
                esel = esel_b
            bmn = ipool.tile([128, RL], FP, tag="bmn", name="bmn")
            _gather(bmn[:R], bm_dram, col, R)
            bp = bmn[:, :].ap[0][0]
            ep = esel[:, :].ap[0][0]
            # one CMUL + store per (mu, window k): fine-grained pipelining
            for mu in range(2):
                for k in range(K):
                    in0 = AP(
                        bmn[:, :].tensor,
                        bmn[:, :].offset + 2 * k,
                        [[bp, R], [1, WF]],
                    )
                    in1 = AP(
                        esel[:, :].tensor,
                        esel[:, :].offset + mu * PW,
                        [[ep, R], [0, 1], [1, WF]],
                    )
                    f = opool.tile([128, WF], FP, tag="f", name="f")
                    _emit_pk(nc, CMUL, f[:R, :], in0, in1)
                    eng = nc.sync if st % 2 == 0 else nc.scalar
                    st += 1
                    eng.dma_start(out=out_d[:, mu, k], in_=f[:R])
                    if bi == 0 and mu == 0 and k == 0:
                        # issue block B's gather now: its packets drain
                        # behind block A's stores instead of its compute
                        esel_b = ipool.tile([128, ERUN], FP, tag="esel", name="esel")
                        _gather(esel_b[:R_B], e_spread, 3, R_B)


# ---------------- host side: cached compiled executable --------------------
_CACHE: dict = {}


def _get_runner(reps: int = 1):
    key = ("runner", reps)
    if key in _CACHE:
        return _CACHE[key]

    import jax
    from jax.sharding import Mesh, PartitionSpec
    from jax.experimental.shard_map import shard_map
    from concourse import bass2jax

    nc = _build_nc(reps)
    bass2jax.install_neuronx_cc_hook()

    partition_name = nc.partition_id_tensor.name if nc.partition_id_tensor else None
    in_names, out_names, out_avals = [], [], []
    for alloc in nc.m.functions[0].allocations:
        if not isinstance(alloc, mybir.MemoryLocationSet):
            continue
        name = alloc.memorylocations[0].name
        if alloc.kind == "ExternalInput":
            if name != partition_name:
                in_names.append(name)
        elif alloc.kind == "ExternalOutput":
            out_names.append(name)
            out_avals.append(
                jax.core.ShapedArray(tuple(alloc.tensor_shape), mybir.dt.np(alloc.dtype))
            )
    n_params = len(in_names)
    n_outs = len(out_avals)
    all_in_names = list(in_names) + list(out_names)
    if partition_name is not None:
        all_in_names.append(partition_name)
    donate = tuple(range(n_params, n_params + n_outs))

    def _body(*args):
        operands = list(args)
        if partition_name is not None:
            operands.append(bass2jax.partition_id_tensor())
        outs = bass2jax._bass_exec_p.bind(
            *operands,
            out_avals=tuple(out_avals),
            in_names=tuple(all_in_names),
            out_names=tuple(out_names),
            lowering_input_output_aliases=(),
            sim_require_finite=False,
            sim_require_nnan=False,
            nc=nc,
        )
        return tuple(outs)

    devices = jax.devices()[:NCORES]
    assert len(devices) == NCORES
    mesh = Mesh(np.asarray(devices), ("core",))
    in_specs = (PartitionSpec("core"),) * (n_params + n_outs)
    out_specs = (PartitionSpec("core"),) * n_outs
    smapped = shard_map(
        _body, mesh=mesh, in_specs=in_specs, out_specs=out_specs, check_rep=False
    )
    sharded = jax.jit(smapped, donate_argnums=donate, keep_unused=True)

    class Runner:
        pass

    R = Runner()
    R.sharded_nodonate = jax.jit(smapped, keep_unused=True)
    R.in_names, R.out_names, R.out_avals, R.mesh = in_names, out_names, out_avals, mesh
    R.nc = nc

    def run(in_maps, device_only=False):
        concat_in = [
            np.concatenate([np.asarray(in_maps[c][nm]) for c in range(NCORES)], axis=0)
            for nm in in_names
        ]
        concat_zeros = [
            np.zeros((NCORES * av.shape[0], *av.shape[1:]), av.dtype) for av in out_avals
        ]
        out_arrs = sharded(*concat_in, *concat_zeros)
        if device_only:
            for o in out_arrs:
                o.block_until_ready()
            return None
        return [
            {
                nm: np.asarray(out_arrs[i]).reshape(NCORES, *out_avals[i].shape)[c]
                for i, nm in enumerate(out_names)
            }
            for c in range(NCORES)
        ]

    R.run = run
    _CACHE[key] = R
    return R


def _make_in_maps(E_real: np.ndarray, E_imag: np.ndarray):
    offs = _CACHE.get("offs")
    if offs is None:
        offs = _CACHE["offs"] = _build_offsets()
    E_real = np.asarray(E_real, dtype=np.float32)
    E_imag = np.asarray(E_imag, dtype=np.float32)
    in_maps = []
    for c in range(NCORES):
        idx = np.arange(c * WLOC - EHALO, (c + 1) * WLOC + EHALO) % W
        planes = np.empty((2 * B, PW), dtype=NPFP)
        for b in range(B):
            for mu in range(NMODES):
                planes[b * 2 + mu, 0::2] = E_real[b, idx, mu].astype(NPFP)
                planes[b * 2 + mu, 1::2] = E_imag[b, idx, mu].astype(NPFP)
        in_maps.append({"e_planes": planes, "offs": offs})
    return in_maps


def _assemble(results) -> np.ndarray:
    out = np.empty((B, W, NMODES, S), dtype=np.complex64)
    for c in range(NCORES):
        wsl = slice(c * WLOC, (c + 1) * WLOC)
        for name, rows, K in (("out_a", ROWS_A, K_A), ("out_b", ROWS_B, K_B)):
            o = np.asarray(results[c][name]).astype(np.float32)  # [R, 2, K, WF]
            cx = o[:, :, :, 0::2] + 1j * o[:, :, :, 1::2]  # [R, 2, K, 2048]
            for j, (b, m, n_hi) in enumerate(rows):
                for k in range(K):
                    n = n_hi - k
                    t = _T_OF.get((m, n))
                    if t is None:
                        continue
                    out[b, wsl, :, t] = cx[j, :, k, :].T
    return out


_T_OF = {mn: i for i, mn in enumerate(TAPS)}


def kernel(E_real: np.ndarray, E_imag: np.ndarray) -> np.ndarray:
    R = _get_runner()
    in_maps = _make_in_maps(E_real, E_imag)
    return _assemble(R.run(in_maps))


def _timed_loop(fn, args, n):
    import time
    import jax

    t0 = time.perf_counter()
    outs = [fn(*args) for _ in range(n)]
    jax.block_until_ready(outs)
    return time.perf_counter() - t0


def _device_args(R, E_real, E_imag):
    import jax
    from jax.sharding import NamedSharding, PartitionSpec

    in_maps = _make_in_maps(E_real, E_imag)
    concat_in = [
        np.concatenate([np.asarray(in_maps[c][nm]) for c in range(NCORES)], axis=0)
        for nm in R.in_names
    ]
    concat_zeros = [
        np.zeros((NCORES * av.shape[0], *av.shape[1:]), av.dtype) for av in R.out_avals
    ]
    shard = NamedSharding(R.mesh, PartitionSpec("core"))
    return [jax.device_put(a, shard) for a in (*concat_in, *concat_zeros)]


def bench(E_real: np.ndarray, E_imag: np.ndarray, iters: int = 40, hi_reps: int = 9):
    import jax

    times = {}
    for reps in (1, hi_reps):
        R = _get_runner(reps)
        args = _device_args(R, E_real, E_imag)
        fn = R.sharded_nodonate
        jax.block_until_ready(fn(*args))
        _timed_loop(fn, args, 3)
        best = min(_timed_loop(fn, args, iters) / iters for _ in range(3))
        times[reps] = best
        print(f"  reps={reps}: per-exec {best * 1e6:.0f} us")
    per_kernel = (times[hi_reps] - times[1]) / (hi_reps - 1)
    return per_kernel, None


if __name__ == "__main__":
    print(f"K_A={K_A} R_A={R_A}  K_B={K_B} R_B={R_B}")
    print("out bytes:", (R_A * K_A + R_B * K_B) * 2 * WF * 2 / 1e6, "MB")
